# revision 1
# baseline (speedup 1.0000x reference)
"""Trainium2 Bass kernel for nn_AutoDecoder (moe_routing).

Reference computation (per full input):
  x: [S=3072, B=32, C=512]; rows s%3==1 are "brick" tokens, s%3==2 are
  "combined" tokens (s%3==0 PAD rows are dead). For each (timestep, batch)
  pair:
    brick:  logits[0:80]    = x_brick @ [Ws|Wc]            (+ biases)
    comb:   h = relu(relu(x_comb @ W1 + b1) @ W2 + b2)
            logits[80:1000] = h @ Wh + bh
  out: [TS=1024, B=32, A=1000]

Strategy: data-parallel over batch (4 batch entries per core, 8 cores),
weights replicated. Per core, tokens are processed in 8 blocks of 512
(ts,b) pairs. x tiles are loaded token-major with a casting DMA
(fp32 HBM -> fp16 SBUF; full HBM read traffic, half SBUF footprint),
transposed on TensorE to feature-major (fp16: full rate + fast weight
load), the 2-layer MLP runs feature-major with fp16 weights and fp32
PSUM accumulation, and the head matmuls use the feature-major
activations as stationary operands to produce token-major logits
(fp32), written back with fully contiguous DMA.

fp16 operands keep 11-bit multiply precision (on par with the PE's
relaxed-fp32 "f32r" mode) while streaming at 1 column/cycle for every
matmul; accumulation is always fp32 in PSUM.
"""
import sys

if "/opt/trn_rl_repo" not in sys.path:
    sys.path.append("/opt/trn_rl_repo")

import numpy as np

import concourse.bass as bass
from concourse import bacc
import concourse.mybir as mybir
import concourse.tile as tile
from concourse.bass import ts
from concourse.bass_utils import run_bass_kernel_spmd
from concourse.masks import make_identity

F32 = mybir.dt.float32
F16 = mybir.dt.float16
RELU = mybir.ActivationFunctionType.Relu

# problem dims (hardcoded; kernel.py must be self-contained)
S, B, C = 3072, 32, 512
TS_ = S // 3                    # 1024 timesteps
NUM_SHAPES, NUM_COLORS, N_COMBINED = 64, 16, 920
NBRICK = NUM_SHAPES + NUM_COLORS  # 80
A = NBRICK + N_COMBINED           # 1000
NCORES = 8
BL = B // NCORES                  # 4 batch entries per core
TT = 128                          # tokens per tok-tile
TPB = TT // BL                    # 32 timesteps per tok-tile
NT = 4                            # tok-tiles per block
BLK_TS = TPB * NT                 # 128 timesteps per block
NBLK = TS_ // BLK_TS              # 8 blocks per core
KC = C // 128                     # 4 contraction chunks

_BUILD_CACHE = {}


def _build():
    if "nc" in _BUILD_CACHE:
        return _BUILD_CACHE["nc"]
    nc = bacc.Bacc("TRN2", target_bir_lowering=False, debug=False)

    x_d = nc.declare_dram_parameter("x", [S, BL, C], F32, isOutput=False)
    w1_d = nc.declare_dram_parameter("w1", [C, C], F16, isOutput=False)
    w2_d = nc.declare_dram_parameter("w2", [C, C], F16, isOutput=False)
    wh_d = nc.declare_dram_parameter("wh", [C, N_COMBINED], F16, isOutput=False)
    wsc_d = nc.declare_dram_parameter("wsc", [C, NBRICK], F16, isOutput=False)
    b1_d = nc.declare_dram_parameter("b1t", [128, KC], F32, isOutput=False)
    b2_d = nc.declare_dram_parameter("b2t", [128, KC], F32, isOutput=False)
    bA_d = nc.declare_dram_parameter("biasA", [128, A], F32, isOutput=False)
    id_d = nc.declare_dram_parameter("ident", [128, 128], F16, isOutput=False)
    out_d = nc.declare_dram_parameter("out", [TS_, BL, A], F32, isOutput=True)

    # x rows by readout name: s = 3*t + r  ->  [r, t, b, c]
    xv = x_d[:].rearrange("(t r) b c -> r t b c", r=3)

    with tile.TileContext(nc) as tc:
        with (
            tc.tile_pool(name="const", bufs=1) as const,
            tc.tile_pool(name="xin", bufs=2) as xin_p,
            tc.tile_pool(name="xt", bufs=2) as xt_p,
            tc.tile_pool(name="h", bufs=2) as h_p,
            tc.tile_pool(name="osb", bufs=4) as o_p,
            tc.tile_pool(name="pst", bufs=2, space=bass.MemorySpace.PSUM) as ps_t,
            tc.tile_pool(name="psh", bufs=2, space=bass.MemorySpace.PSUM) as ps_h,
            tc.tile_pool(name="psc", bufs=2, space=bass.MemorySpace.PSUM) as ps_c,
        ):
            # ---- constants / weights (all fp16, DMA'd directly) ----
            # HAM warmup: ~4us of dummy matmuls at t=0 (on a memset scratch,
            # no DMA dependency) so the PE clock gate is already released
            # (K=8/8) when the real work arrives.
            warm_src = const.tile([128, 128], F16, tag="warm")
            nc.vector.memset(warm_src[:], 0.0)
            warm = ps_h.tile([128, 512], F32, tag="hps")
            for _ in range(144):
                nc.tensor.matmul(warm[:, 0:128], warm_src[:], warm_src[:])
            # pre-fire the one-time ACT activation-table load so the first
            # real relu doesn't pay ~1.3us for it
            warm_act = const.tile([128, 1], F32, tag="warmact")
            nc.scalar.activation(warm_act[0:1, 0:1], warm_src[0:1, 0:1], RELU)
            ident = const.tile([128, 128], F16, tag="ident")
            nc.scalar.dma_start(ident[:], id_d[:, :])
            # only w1/w2 are needed in the first ~15us; everything else is
            # deferred below so block 0's x loads get the DMA bandwidth
            w1_sb = []
            w2_sb = []
            wh_sb = []
            wsc_sb = []
            for k in range(KC):
                for name, dram, width, out_list in (
                    ("w1", w1_d, C, w1_sb),
                    ("w2", w2_d, C, w2_sb),
                ):
                    t = const.tile([128, width], F16, tag=f"{name}_{k}")
                    nc.sync.dma_start(t[:], dram[ts(k, 128), :])
                    out_list.append(t)

            def load_deferred_consts():
                for k in range(KC):
                    for name, dram, width, out_list in (
                        ("wh", wh_d, N_COMBINED, wh_sb),
                        ("wsc", wsc_d, NBRICK, wsc_sb),
                    ):
                        t = const.tile([128, width], F16, tag=f"{name}_{k}")
                        nc.sync.dma_start(t[:], dram[ts(k, 128), :])
                        out_list.append(t)
                b1_sb = const.tile([128, KC], F32, tag="b1")
                nc.sync.dma_start(b1_sb[:], b1_d[:, :])
                b2_sb = const.tile([128, KC], F32, tag="b2")
                nc.sync.dma_start(b2_sb[:], b2_d[:, :])
                bA_sb = const.tile([128, A], F32, tag="biasA")
                nc.sync.dma_start(bA_sb[:], bA_d[:, :])
                return b1_sb, b2_sb, bA_sb

            # ---- main loop over blocks of tok-tiles (128 tokens each) ----
            # ramp-up/ramp-down schedule: small blocks first (fast pipeline
            # fill, early HAM release) and last (short drain tail).
            # Heads ("finals") for block i are emitted during block i+1 so
            # the DVE bias-adds of block i never head-of-line-block block
            # i+1's transpose copies in the strict-FIFO DVE queue.
            def finals(pb):
                for t in range(pb["nt"]):
                    pco = ps_c.tile([128, 1024], F32, tag="combo")
                    for k in range(KC):
                        lhs = pb["h2"][k][:, ts(t, 128)]
                        nc.tensor.matmul(
                            pco[:, 0:512],
                            lhs,
                            wh_sb[k][:, 0:512],
                            start=(k == 0),
                            stop=(k == KC - 1),
                        )
                        nc.tensor.matmul(
                            pco[:, 512:N_COMBINED],
                            lhs,
                            wh_sb[k][:, 512:N_COMBINED],
                            start=(k == 0),
                            stop=(k == KC - 1),
                        )
                    for k in range(KC):
                        nc.tensor.matmul(
                            pco[:, N_COMBINED:A],
                            pb["xt"][(0, k)][:, ts(t, 128)],
                            wsc_sb[k][:],
                            start=(k == 0),
                            stop=(k == KC - 1),
                        )
                    ot = o_p.tile([128, A], F32, tag="osb")
                    nc.vector.tensor_add(
                        ot[:, NBRICK:A], pco[:, 0:N_COMBINED], bA_sb[:, NBRICK:A]
                    )
                    nc.vector.tensor_add(
                        ot[:, 0:NBRICK], pco[:, N_COMBINED:A], bA_sb[:, 0:NBRICK]
                    )
                    nc.sync.dma_start(
                        out_d[pb["t0"] + t * TPB : pb["t0"] + (t + 1) * TPB, :, :],
                        ot[:],
                    )

            sched = [2] + [4] * 7 + [1, 1]
            assert sum(sched) == TS_ // TPB
            ti0 = 0
            pending = None
            for nt in sched:
                t0 = ti0 * TPB
                W_ = nt * TT  # tokens per name in this block
                # token-major casting loads: [128 tokens, C] fp16;
                # name 0=brick(r=1), 1=comb(r=2)
                # comb (ni=1) loads/transposes first: the MLP needs them
                # right away, the brick head not until the next block
                xin = {}
                for ni in (1, 0):
                    for t in range(nt):
                        tl = xin_p.tile([TT, C], F16, tag=f"xin{ni}{(ti0 + t) % 4}")
                        nc.gpsimd.dma_start(
                            tl[:],
                            xv[1 + ni, t0 + t * TPB : t0 + (t + 1) * TPB, :, :],
                        )
                        xin[(ni, t)] = tl
                if ti0 == 0:
                    b1_sb, b2_sb, bA_sb = load_deferred_consts()
                # transpose to feature-major, per (name, c-chunk)
                xt = {}
                for ni in (1, 0):
                    for j in range(KC):
                        pst = ps_t.tile([128, W_], F16, tag="trps")
                        for t in range(nt):
                            nc.tensor.transpose(
                                pst[:, ts(t, 128)],
                                xin[(ni, t)][:, ts(j, 128)],
                                ident[:],
                            )
                        sb = xt_p.tile([128, W_], F16, tag=f"xt{ni}{j}")
                        nc.vector.tensor_copy(sb[:], pst[:])
                        xt[(ni, j)] = sb

                # previous block's heads (see note above)
                if pending is not None:
                    finals(pending)

                # comb MLP layer 1: h1T[m] = relu(W1[:,m-chunk].T @ xT + b1)
                h1 = []
                for m in range(KC):
                    ph = ps_h.tile([128, W_], F32, tag="hps")
                    for k in range(KC):
                        nc.tensor.matmul(
                            ph[:],
                            w1_sb[k][:, ts(m, 128)],
                            xt[(1, k)][:],
                            start=(k == 0),
                            stop=(k == KC - 1),
                        )
                    hs = h_p.tile([128, W_], F16, tag=f"h1_{m}")
                    nc.scalar.activation(
                        hs[:], ph[:], RELU, bias=b1_sb[:, m : m + 1], scale=1.0
                    )
                    h1.append(hs)
                # layer 2
                h2 = []
                for m in range(KC):
                    ph = ps_h.tile([128, W_], F32, tag="hps")
                    for k in range(KC):
                        nc.tensor.matmul(
                            ph[:],
                            w2_sb[k][:, ts(m, 128)],
                            h1[k][:],
                            start=(k == 0),
                            stop=(k == KC - 1),
                        )
                    hs = h_p.tile([128, W_], F16, tag=f"h2_{m}")
                    nc.scalar.activation(
                        hs[:], ph[:], RELU, bias=b2_sb[:, m : m + 1], scale=1.0
                    )
                    h2.append(hs)

                pending = {"h2": h2, "xt": xt, "t0": t0, "nt": nt}
                ti0 += nt
            finals(pending)

    nc.compile()
    _BUILD_CACHE["nc"] = nc
    return nc


def _prepare_inputs(inputs):
    """Host-side prep: validate/normalize routing, shard over batch,
    replicate weights. Returns in_maps for the 8 cores."""
    x = np.ascontiguousarray(np.asarray(inputs["x"], dtype=np.float32))
    readout_x = np.asarray(inputs["readout_x"], dtype=np.int32)
    W1 = np.asarray(inputs["W1"], dtype=np.float32)
    W2 = np.asarray(inputs["W2"], dtype=np.float32)
    Wh = np.asarray(inputs["Wh"], dtype=np.float32)
    Ws = np.asarray(inputs["Ws"], dtype=np.float32)
    Wc = np.asarray(inputs["Wc"], dtype=np.float32)
    b1 = np.asarray(inputs["b1"], dtype=np.float32)
    b2 = np.asarray(inputs["b2"], dtype=np.float32)
    bh = np.asarray(inputs["bh"], dtype=np.float32)
    bs = np.asarray(inputs["bs"], dtype=np.float32)
    bc = np.asarray(inputs["bc"], dtype=np.float32)

    # The kernel hardcodes the cyclic PAD/brick/comb routing. If the actual
    # readout pattern differs, permute x on the host so the device sees the
    # canonical layout (mirrors jnp.nonzero(..., size=ntok) semantics).
    ntok = TS_ * B
    rf = readout_x.reshape(-1)
    canonical = np.array_equal(
        readout_x, np.broadcast_to((np.arange(S, dtype=np.int32) % 3)[:, None], (S, B))
    )
    if not canonical:
        xf = x.reshape(S * B, C)
        xc = np.zeros_like(x).reshape(S * B, C)
        for name_idx in (1, 2):
            idx = np.nonzero(rf == name_idx)[0]
            if idx.shape[0] < ntok:
                idx = np.pad(idx, (0, ntok - idx.shape[0]))
            else:
                idx = idx[:ntok]
            tgt = (3 * (np.arange(ntok) // B) + name_idx) * B + (np.arange(ntok) % B)
            xc[tgt] = xf[idx]
        x = xc.reshape(S, B, C)

    Wsc = np.ascontiguousarray(np.concatenate([Ws, Wc], axis=1).astype(np.float16))
    W1h = np.ascontiguousarray(W1.astype(np.float16))
    W2h = np.ascontiguousarray(W2.astype(np.float16))
    Whh = np.ascontiguousarray(Wh.astype(np.float16))
    b1t = np.ascontiguousarray(b1.reshape(KC, 128).T)
    b2t = np.ascontiguousarray(b2.reshape(KC, 128).T)
    biasA = np.concatenate([bs, bc, bh])
    biasA_b = np.ascontiguousarray(np.broadcast_to(biasA, (128, A)))
    ident = np.eye(128, dtype=np.float16)

    in_maps = []
    for c in range(NCORES):
        in_maps.append(
            {
                "x": np.ascontiguousarray(x[:, c * BL : (c + 1) * BL, :]),
                "w1": W1h,
                "w2": W2h,
                "wh": Whh,
                "wsc": Wsc,
                "b1t": b1t,
                "b2t": b2t,
                "biasA": biasA_b,
                "ident": ident,
            }
        )
    return in_maps


def _run(inputs, trace=False, trace_kwargs=None):
    nc = _build()
    in_maps = _prepare_inputs(inputs)
    res = run_bass_kernel_spmd(
        nc,
        in_maps,
        list(range(NCORES)),
        trace=trace,
        **(trace_kwargs or {}),
    )
    out = np.empty((TS_, B, A), dtype=np.float32)
    for c in range(NCORES):
        out[:, c * BL : (c + 1) * BL, :] = res.results[c]["out"]
    return out, res


def kernel(**inputs) -> np.ndarray:
    out, _ = _run(inputs, trace=False)
    return out


if __name__ == "__main__":
    nc = _build()
    print("built OK")

